# revision 1
# baseline (speedup 1.0000x reference)
"""Trainium2 Bass kernel for nn_MetricLoss (segment_reduce / discriminative loss).

Reference math (K=32 labels, D=16):
  cents[s,k,:]  = mean of embeddings of sample s where label==k
  push[s]       = sum_{k<j} relu(0.25 - L1(c_sk, c_sj))^2 / 496
  pull[s]       = mean over ALL B*H*W pixels p of  L1(e_p, c_s,label_p)^2
  loss          = mean_s (push[s] + 0.1 * pull[s])

Strategy (8 cores, two launches, pixel-major layout [128 part x 576 cols]):
  Launch A: per-core partial centroid sums+counts.
    - one-hot(labels) in bf16 via DVE is_equal
    - PE: 84 groups; weights = 7 pixel-tiles of [emb(16) ; ones(1)] = 119 cols
      (padded to 128), rhs = the 7 tiles' one-hot [128, 224]; accumulated into
      a single PSUM bank; diagonal blocks hold per-tile-class sums+counts.
    - host sums blocks across groups/cores -> cents [4,32,16]
  Launch B: pull + push.
    - onehotT4 [128=(strip4,k32), 18432] via DVE tensor_scalar is_equal (4x)
    - gather: per pixel-tile MM with lhsT = onehotT4 strip slice [32,128],
      rhs = centsT [32,64] -> psum [128 px, 64=(b,d)] = gathered centroids
    - DVE: diff = psum - emb (b-broadcast), |.|-reduce over d -> dist [128,4,576]
    - DVE: per-b sum of dist^2 -> pacc [128,4] -> host reduce
    - push computed redundantly per core from tiny cent tables.
"""

import numpy as np
import ml_dtypes

import concourse.bass as bass
import concourse.bacc as bacc
import concourse.mybir as mybir
from concourse.tile import TileContext
from concourse.bass_utils import run_bass_kernel_spmd

BF16 = ml_dtypes.bfloat16
F32 = np.float32

# problem constants (hardcoded per contract)
B, H, W, D, K = 4, 384, 384, 16, 32
NCORES = 8
NPIX_TOT = B * H * W              # 589824
NPIX = NPIX_TOT // NCORES         # 73728 per core
P = 128                           # partitions
TC = NPIX // P                    # 576 pixel columns per partition
TCP = 588                         # padded to 7*84 for launch A grouping
NG = TCP // 7                     # 84 weight groups
GW = 7 * 17                       # 119 weight cols per group
WCOLS = GW * (NG - 1) + 128       # 10005 -> pad
WCOLS_PAD = 10016
NB = TC // 8                      # 72 gather batches (8 tiles each)
QS = TC // 4                      # 144 tiles per strip
LAB_PAD = 100.0                   # pad label (!= any of 0..31)

PUSH_MARGIN = 0.25
PUSH_W = 1.0
PULL_W = 0.1
NCMP = K * (K - 1) / 2.0

_built = {}


def _build_launch_a():
    nc = bacc.Bacc("TRN2", target_bir_lowering=False, debug=False)
    bf = mybir.dt.bfloat16
    f32 = mybir.dt.float32

    emb17 = nc.dram_tensor("emb17", [P, WCOLS_PAD], bf, kind="ExternalInput")
    labels = nc.dram_tensor("labels", [P, TCP], bf, kind="ExternalInput")
    outA = nc.dram_tensor("outA", [P, 224], f32, kind="ExternalOutput")

    with TileContext(nc) as tc:
        with (
            tc.tile_pool(name="sbuf", bufs=1) as pool,
            tc.tile_pool(name="psum", bufs=1, space="PSUM") as psum_pool,
        ):
            emb_sb = pool.tile([P, WCOLS_PAD], bf)
            lab_sb = pool.tile([P, TCP], bf)
            onehot = pool.tile([P, K, TCP], bf)  # k-major: 4x-mode generation

            nc.sync.dma_start(out=lab_sb[:], in_=labels.ap())
            # emb17 in 4 chunks for DMA/PE overlap
            NCH = 4
            ch = WCOLS_PAD // NCH
            for i in range(NCH):
                nc.sync.dma_start(
                    out=emb_sb[:, i * ch : (i + 1) * ch],
                    in_=emb17.ap()[:, i * ch : (i + 1) * ch],
                )

            # one-hot: per-k tensor_scalar is_equal (single-src bf16 -> 4x mode)
            half = TCP // 2
            for h in range(2):
                sl = slice(h * half, (h + 1) * half)
                for k in range(K):
                    nc.vector.tensor_scalar(
                        out=onehot[:, k, sl],
                        in0=lab_sb[:, sl],
                        scalar1=float(k),
                        scalar2=None,
                        op0=mybir.AluOpType.is_equal,
                    )

            ps = psum_pool.tile([P, 7, K], mybir.dt.float32)
            for g in range(NG):
                nc.tensor.matmul(
                    ps[:],
                    emb_sb[:, GW * g : GW * g + 128],
                    onehot[:, :, 7 * g : 7 * g + 7].rearrange("p k t -> p t k"),
                    start=(g == 0),
                    stop=(g == NG - 1),
                )

            evac = pool.tile([P, 7 * K], f32)
            nc.vector.tensor_copy(out=evac[:], in_=ps[:].rearrange("p a b -> p (a b)"))
            nc.sync.dma_start(out=outA.ap(), in_=evac[:])
    nc.compile()
    return nc


def _build_launch_b():
    nc = bacc.Bacc("TRN2", target_bir_lowering=False, debug=False)
    bf = mybir.dt.bfloat16
    f32 = mybir.dt.float32

    emb16 = nc.dram_tensor("emb16", [P, TC * D], bf, kind="ExternalInput")
    lab4 = nc.dram_tensor("lab4", [P, QS * P], bf, kind="ExternalInput")
    iotaT = nc.dram_tensor("iotaT", [P, 1], f32, kind="ExternalInput")
    centsT = nc.dram_tensor("centsT", [P, 64], bf, kind="ExternalInput")
    cpp = nc.dram_tensor("cpp", [P, D], bf, kind="ExternalInput")
    cjd = nc.dram_tensor("cjd", [P, K * D], bf, kind="ExternalInput")
    triu = nc.dram_tensor("triu", [P, K], bf, kind="ExternalInput")
    pacc_d = nc.dram_tensor("pacc", [P, 4], f32, kind="ExternalOutput")
    pushp_d = nc.dram_tensor("pushp", [P, 1], f32, kind="ExternalOutput")

    with TileContext(nc) as tc:
        with (
            tc.tile_pool(name="sbuf", bufs=1) as pool,
            tc.tile_pool(name="work", bufs=3) as wpool,
            tc.tile_pool(name="psum", bufs=2, space="PSUM") as psum_pool,
        ):
            emb_sb = pool.tile([P, TC, D], bf)
            lab4_sb = pool.tile([P, QS * P], bf)
            iotaT_sb = pool.tile([P, 1], f32)
            centsT_sb = pool.tile([P, 64], bf)
            cpp_sb = pool.tile([P, D], bf)
            cjd_sb = pool.tile([P, K, D], bf)
            triu_sb = pool.tile([P, K], bf)
            oh4 = pool.tile([P, QS * P], bf)
            dist = pool.tile([P, TC, 4], bf)  # t-major, b-inner (2x-mode APs)
            pacc = pool.tile([P, 4], f32)
            pushp = pool.tile([P, 1], f32)

            nc.sync.dma_start(out=iotaT_sb[:], in_=iotaT.ap())
            nc.sync.dma_start(out=centsT_sb[:], in_=centsT.ap())
            nc.sync.dma_start(out=cpp_sb[:], in_=cpp.ap())
            nc.sync.dma_start(out=cjd_sb[:], in_=cjd.ap().rearrange("p (a b) -> p a b", b=D))
            nc.sync.dma_start(out=triu_sb[:], in_=triu.ap())

            NCH = 4
            ech = TC // NCH
            for i in range(NCH):
                nc.sync.dma_start(
                    out=emb_sb[:, i * ech : (i + 1) * ech, :],
                    in_=emb16.ap().rearrange("p (t d) -> p t d", d=D)[
                        :, i * ech : (i + 1) * ech, :
                    ],
                )
            lch = (QS * P) // NCH
            for i in range(NCH):
                nc.sync.dma_start(
                    out=lab4_sb[:, i * lch : (i + 1) * lch],
                    in_=lab4.ap()[:, i * lch : (i + 1) * lch],
                )

            # one-hot (transposed, 4 strips) via tensor_scalar is_equal (4x mode)
            NOH = 12
            oc = (QS * P) // NOH
            for i in range(NOH):
                sl = slice(i * oc, (i + 1) * oc)
                nc.vector.tensor_scalar(
                    out=oh4[:, sl],
                    in0=lab4_sb[:, sl],
                    scalar1=iotaT_sb[:, 0:1],
                    scalar2=None,
                    op0=mybir.AluOpType.is_equal,
                )

            # gather + pull distance; superbatches of 32 tiles, one PSUM bank
            # per strip (concurrent row-strip MMs must hit distinct banks).
            # MM emission interleaves strips so next LDW overlaps current MM.
            NSB = TC // 32
            for sb in range(NSB):
                t0 = 32 * sb
                pss = [
                    psum_pool.tile(
                        [P, 8, 4, D], mybir.dt.float32, tag=f"ps{s}",
                        name=f"ps{s}_{sb}",
                    )
                    for s in range(4)
                ]
                for j in range(8):
                    for s in range(4):
                        q = 8 * sb + j
                        nc.tensor.matmul(
                            pss[s][:, j, :, :].rearrange("p a b -> p (a b)"),
                            oh4[32 * s : 32 * s + 32, P * q : P * (q + 1)],
                            centsT_sb[32 * s : 32 * s + 32, :],
                            start=True,
                            stop=True,
                            tile_position=(32 * s, 0),
                        )
                for s in range(4):
                    gev = wpool.tile([P, 8, 4, D], bf, tag=f"gev{s}")
                    nc.scalar.copy(out=gev[:], in_=pss[s][:])
                    diff = wpool.tile([P, 8, 4, D], bf, tag=f"diff{s}")
                    nc.vector.tensor_tensor(
                        out=diff[:],
                        in0=gev[:],
                        in1=emb_sb[:, t0 + s : t0 + s + 29 : 4, :]
                        .unsqueeze(2)
                        .broadcast_to([P, 8, 4, D]),
                        op=mybir.AluOpType.subtract,
                    )
                    with nc.allow_low_precision("dist in bf16; error averages out"):
                        nc.vector.tensor_reduce(
                            out=dist[:, t0 + s : t0 + s + 29 : 4, :],
                            in_=diff[:],
                            axis=mybir.AxisListType.X,
                            op=mybir.AluOpType.add,
                            apply_absolute_value=True,
                        )

            # pull partial: pacc[p, b] = sum_t dist^2
            sq = pool.tile([P, TC, 4], f32)
            nc.vector.tensor_tensor(
                out=sq[:], in0=dist[:], in1=dist[:], op=mybir.AluOpType.mult
            )
            nc.vector.tensor_reduce(
                out=pacc[:],
                in_=sq[:].rearrange("p t b -> p b t"),
                axis=mybir.AxisListType.X,
                op=mybir.AluOpType.add,
            )
            nc.sync.dma_start(out=pacc_d.ap(), in_=pacc[:])

            # push (tiny, redundant per core): partitions p=(b,k)
            pd_diff = pool.tile([P, K, D], bf)
            nc.vector.tensor_tensor(
                out=pd_diff[:],
                in0=cpp_sb[:].unsqueeze(1).broadcast_to([P, K, D]),
                in1=cjd_sb[:],
                op=mybir.AluOpType.subtract,
            )
            pd = pool.tile([P, K], f32)
            nc.vector.tensor_reduce(
                out=pd[:],
                in_=pd_diff[:],
                axis=mybir.AxisListType.X,
                op=mybir.AluOpType.add,
                apply_absolute_value=True,
            )
            # relu(margin - d)^2 == min(d - margin, 0)^2
            m = pool.tile([P, K], f32)
            nc.vector.tensor_scalar(
                out=m[:],
                in0=pd[:],
                scalar1=PUSH_MARGIN,
                scalar2=0.0,
                op0=mybir.AluOpType.subtract,
                op1=mybir.AluOpType.min,
            )
            msq = pool.tile([P, K], f32)
            nc.vector.tensor_tensor(
                out=msq[:], in0=m[:], in1=m[:], op=mybir.AluOpType.mult
            )
            msqm = pool.tile([P, K], f32)
            nc.vector.tensor_tensor(
                out=msqm[:], in0=msq[:], in1=triu_sb[:], op=mybir.AluOpType.mult
            )
            nc.vector.tensor_reduce(
                out=pushp[:],
                in_=msqm[:],
                axis=mybir.AxisListType.X,
                op=mybir.AluOpType.add,
            )
            nc.sync.dma_start(out=pushp_d.ap(), in_=pushp[:])
    nc.compile()
    return nc


def _get(name):
    if name not in _built:
        if name == "A":
            _built[name] = _build_launch_a()
        else:
            _built[name] = _build_launch_b()
    return _built[name]


def _prep_a(emb_flat, lab_flat):
    """emb_flat [NPIX_TOT, D] f32, lab_flat [NPIX_TOT] i32 -> per-core in_maps."""
    in_maps = []
    for c in range(NCORES):
        e = emb_flat[c * NPIX : (c + 1) * NPIX].astype(BF16).reshape(P, TC, D)
        l = lab_flat[c * NPIX : (c + 1) * NPIX].reshape(P, TC)
        e17 = np.zeros((P, TCP, 17), dtype=BF16)
        e17[:, :TC, :D] = e
        e17[:, :, D] = BF16(1.0)
        w = np.zeros((P, WCOLS_PAD), dtype=BF16)
        w[:, : TCP * 17] = e17.reshape(P, TCP * 17)
        lb = np.full((P, TCP), LAB_PAD, dtype=BF16)
        lb[:, :TC] = l.astype(BF16)
        in_maps.append({"emb17": w, "labels": lb})
    return in_maps


def _reduce_a(results):
    """outA [8][P, 224] -> cents [B, K, D] float64, counts [B, K]."""
    sums = np.zeros((B, K, D), dtype=np.float64)
    cnts = np.zeros((B, K), dtype=np.float64)
    for c in range(NCORES):
        o = results[c]["outA"].astype(np.float64).reshape(P, 7, K)
        s = c // 2
        for j in range(7):
            blk = o[17 * j : 17 * j + 17, j, :]  # [17, K]
            sums[s] += blk[:D].T  # [K, D]
            cnts[s] += blk[D]
    cents = sums / np.maximum(cnts, 1.0)[:, :, None]
    cents = np.where(cnts[:, :, None] > 0, cents, 0.0)
    return cents, cnts


def _prep_b(emb_flat, lab_flat, cents):
    iotaT = (np.arange(P, dtype=F32) % K).astype(F32).reshape(P, 1)
    centsT = np.zeros((P, 64), dtype=BF16)
    cb = cents.astype(F32)  # [B, K, D]
    for s in range(4):
        # centsT[32s+k, 16b+d] = cents[b, k, d]
        centsT[32 * s : 32 * s + 32, :] = (
            cb.transpose(1, 0, 2).reshape(K, 64).astype(BF16)
        )
    cpp = cb.reshape(P, D).astype(BF16)  # p = 32b + k
    cjd = np.zeros((P, K * D), dtype=BF16)
    for b in range(4):
        cjd[32 * b : 32 * b + 32, :] = np.broadcast_to(
            cb[b].reshape(1, K * D), (K, K * D)
        ).astype(BF16)
    triu = np.zeros((P, K), dtype=BF16)
    kk = np.arange(K)
    for b in range(4):
        triu[32 * b : 32 * b + 32, :] = (kk[None, :] > kk[:, None]).astype(BF16)

    in_maps = []
    for c in range(NCORES):
        e = emb_flat[c * NPIX : (c + 1) * NPIX].astype(BF16).reshape(P, TC, D)
        l = lab_flat[c * NPIX : (c + 1) * NPIX].reshape(P, TC)  # [m, tau]
        lab4 = np.zeros((P, QS * P), dtype=BF16)
        for s in range(4):
            a = l[:, s::4]  # [m, q]
            lab4[32 * s : 32 * s + 32, :] = np.broadcast_to(
                a.T.reshape(1, QS * P), (K, QS * P)
            ).astype(BF16)
        in_maps.append(
            {
                "emb16": e.reshape(P, TC * D),
                "lab4": lab4,
                "iotaT": iotaT.copy(),
                "centsT": centsT.copy(),
                "cpp": cpp.copy(),
                "cjd": cjd.copy(),
                "triu": triu.copy(),
            }
        )
    return in_maps


def run_launches(embeddings, labels, trace=False, trace_kwargs=None):
    """Returns (loss_scalar, resA, resB) — resA/resB are BassKernelResults."""
    emb_flat = np.ascontiguousarray(np.asarray(embeddings), dtype=F32).reshape(
        NPIX_TOT, D
    )
    lab_flat = np.ascontiguousarray(np.asarray(labels), dtype=np.int32).reshape(
        NPIX_TOT
    )
    core_ids = list(range(NCORES))

    kwA = dict(trace=trace, **(trace_kwargs or {}))
    resA = run_bass_kernel_spmd(_get("A"), _prep_a(emb_flat, lab_flat), core_ids, **kwA)
    cents, _ = _reduce_a(resA.results)

    resB = run_bass_kernel_spmd(
        _get("B"), _prep_b(emb_flat, lab_flat, cents), core_ids, **kwA
    )
    pull = np.zeros(4, dtype=np.float64)
    for c in range(NCORES):
        pull += resB.results[c]["pacc"].astype(np.float64).sum(axis=0)
    pull /= NPIX_TOT

    pushp = resB.results[0]["pushp"].astype(np.float64).reshape(4, K).sum(axis=1)
    push = pushp / NCMP

    loss = np.mean(PUSH_W * push + PULL_W * pull)
    return np.array(loss, dtype=F32), resA, resB


def kernel(embeddings, labels):
    loss, _, _ = run_launches(embeddings, labels, trace=False)
    return loss



# revision 13
# speedup vs baseline: 1.8327x; 1.8327x over previous
"""Trainium2 Bass kernel for nn_MetricLoss (segment_reduce / discriminative loss).

Reference math (B=4 samples, K=32 labels, D=16, H=W=384):
  cents[s,k,:]  = mean of embeddings of sample s where label==k
  push[s]       = sum_{k<j} relu(0.25 - L1(c_sk, c_sj))^2 / 496
  pull[s]       = mean over ALL B*H*W pixels p of  L1(e_p, c_s,label_p)^2
  loss          = mean_s (push[s] + 0.1 * pull[s])

v2 design — host sorts pixels by label so that the centroid needed by any
on-chip operation is a per-partition constant; this removes every gather.

  Launch A (centroid partial sums; "lane" transposed layout):
    partition p = 16*j + d holds the d-th embedding component of pixels whose
    label k has k%8 == j, grouped into 16 equal segments (q=k//8, b) on the
    free axis (zero padded).  One tensor_scalar(mult 1, accum_out) per segment
    runs at DVE 4x and yields partial sums per (b,k,d).  Host reduces over
    cores, forms centroids, and computes the tiny push term exactly in f64.

  Launch B (pull term; pixel-partition d-major layout + PE Gram):
    partition p holds pixels of label k(p) = p//4; emb2[p, d, t] is d-major.
    |e - c_{s,k(p),d}| for each of the 4 sample-centroid-sets s:
      one fused tensor_scalar(subtract, abs_max 0) per (s,d)  [DVE 4x]
      (odd d offloaded to ACT as abs(e + (-c)) with per-partition bias).
    d-pairs summed (tensor_tensor add, 2x) -> h_s[p, e=0..7, t].
    sum_t dist^2 = sum over the 8x8 diagonal blocks of the Gram matrix of
    h-chunks [128 pixels x (16 t x 8 e)], accumulated on the PE in PSUM over
    all t-chunks; a diag-block-mask tensor_tensor_reduce gives pacc[p, s].
    Host: pull_s = (sum pacc - exact zero-pad correction) / N.
"""

import numpy as np
import ml_dtypes

import concourse.bass as bass
import concourse.bacc as bacc
import concourse.mybir as mybir
from concourse.tile import TileContext
from concourse.bass_utils import run_bass_kernel_spmd

BF16 = ml_dtypes.bfloat16
F32 = np.float32

# problem constants (hardcoded per contract)
B, H, W, D, K = 4, 384, 384, 16, 32
NCORES = 8
NPIX_TOT = B * H * W            # 589824
P = 128                         # partitions

PUSH_MARGIN = 0.25
PUSH_W = 1.0
PULL_W = 0.1
NCMP = K * (K - 1) / 2.0

_built = {}


# --------------------------------------------------------------------------
# device programs
# --------------------------------------------------------------------------

def _build_a(SbA):
    """Centroid partial sums.  embA [128=(16j+d), 16*SbA] bf16 -> accA [128,16] f32."""
    nc = bacc.Bacc("TRN2", target_bir_lowering=False, debug=False)
    bf = mybir.dt.bfloat16
    f32 = mybir.dt.float32
    FREE = 16 * SbA

    embA_d = nc.dram_tensor("embA", [P, FREE], bf, kind="ExternalInput")
    accA_d = nc.dram_tensor("accA", [P, 16], f32, kind="ExternalOutput")

    with TileContext(nc) as tc:
        with (
            tc.tile_pool(name="sbuf", bufs=1) as pool,
            tc.tile_pool(name="scr", bufs=2) as spool,
        ):
            emb_sb = pool.tile([P, 16, SbA], bf)
            accA = pool.tile([P, 16], f32)
            NCH = 4
            src = embA_d.ap().rearrange("p (s t) -> p s t", t=SbA)
            for c in range(NCH):
                nc.sync.dma_start(
                    out=emb_sb[:, 4 * c : 4 * (c + 1), :],
                    in_=src[:, 4 * c : 4 * (c + 1), :],
                )
            # segment sums: DVE tensor_reduce and ACT copy+accum split the 16
            # segments (both 1x; DMA-bound overall).  Alternate so both
            # engines advance with the arriving DMA chunks.
            for seg in range(16):
                if seg % 2 == 0:
                    nc.vector.tensor_reduce(
                        out=accA[:, seg : seg + 1],
                        in_=emb_sb[:, seg, :],
                        axis=mybir.AxisListType.X,
                        op=mybir.AluOpType.add,
                    )
                else:
                    scr = spool.tile([P, SbA], bf, tag="scr")
                    nc.scalar.activation(
                        out=scr[:],
                        in_=emb_sb[:, seg, :],
                        func=mybir.ActivationFunctionType.Copy,
                        accum_out=accA[:, seg : seg + 1],
                    )
            nc.sync.dma_start(out=accA_d.ap(), in_=accA[:])
    nc.compile()
    return nc


def _build_b(S):
    """Pull term.  emb2 [128, 16*S] bf16 (d-major), embE [128, S] bf16
    (host-precomputed sum_d e), centq [128,64] f32 (c per (s,d)),
    csum [128,4] f32 (sum_d c per s) -> pacc [128, 4] f32.

    Uses |x| = x - 2*min(x,0):
      dist_s = (sum_d e - sum_d c) - 2 * sum_d min(e_d - c_d, 0)
    min-diffs: rows 0-7 on DVE (ts subtract+min at 4x, values m<=0), rows
    8-15 on ACT (relu(c-e) = -m >= 0); a 4-level adder tree (signs resolved
    by add/sub choices), then dist via scalar_tensor_tensor and ACT
    Square+accum_out."""
    nc = bacc.Bacc("TRN2", target_bir_lowering=False, debug=False)
    bf = mybir.dt.bfloat16
    f32 = mybir.dt.float32

    emb2_d = nc.dram_tensor("emb2", [P, 16 * S], bf, kind="ExternalInput")
    embE_d = nc.dram_tensor("embE", [P, S], bf, kind="ExternalInput")
    centq_d = nc.dram_tensor("centq", [P, 64], f32, kind="ExternalInput")
    csum_d = nc.dram_tensor("csum", [P, 4], f32, kind="ExternalInput")
    pacc_d = nc.dram_tensor("pacc", [P, 4], f32, kind="ExternalOutput")

    with TileContext(nc) as tc:
        with (
            tc.tile_pool(name="sbuf", bufs=1) as pool,
            tc.tile_pool(name="apool", bufs=2) as apool,
            tc.tile_pool(name="hpool", bufs=2) as hpool,
            tc.tile_pool(name="spool", bufs=2) as spool,
        ):
            emb2 = pool.tile([P, 16, S], bf)
            embE = pool.tile([P, S], bf)
            centq = pool.tile([P, 64], f32)
            csum = pool.tile([P, 4], f32)
            pacc = pool.tile([P, 4], f32)

            nc.sync.dma_start(out=centq[:], in_=centq_d.ap())
            nc.sync.dma_start(out=csum[:], in_=csum_d.ap())
            nc.sync.dma_start(out=embE[:], in_=embE_d.ap())
            src = emb2_d.ap().rearrange("p (d t) -> p d t", t=S)
            for c in range(8):
                nc.sync.dma_start(
                    out=emb2[:, 2 * c : 2 * c + 2, :],
                    in_=src[:, 2 * c : 2 * c + 2, :],
                )

            for s in range(4):
                a_s = apool.tile([P, 16, S], bf, tag="a")
                for d in range(16):
                    col = 16 * s + d
                    if d < 8:  # DVE: m_d = min(e-c, 0)  (<= 0)
                        nc.vector.tensor_scalar(
                            out=a_s[:, d, :],
                            in0=emb2[:, d, :],
                            scalar1=centq[:, col : col + 1],
                            scalar2=0.0,
                            op0=mybir.AluOpType.subtract,
                            op1=mybir.AluOpType.min,
                        )
                    else:  # ACT: q_d = relu(c-e) = -m_d  (>= 0)
                        nc.scalar.activation(
                            out=a_s[:, d, :],
                            in_=emb2[:, d, :],
                            func=mybir.ActivationFunctionType.Relu,
                            bias=centq[:, col : col + 1],
                            scale=-1.0,
                        )
                # level 1: rows 0-3 carry +(m+m'), rows 4-7 carry -(m+m')
                h1 = hpool.tile([P, 8, S], bf, tag="h1")
                nc.vector.tensor_tensor(
                    out=h1[:, 0:4, :], in0=a_s[:, 0:8:2, :], in1=a_s[:, 1:8:2, :],
                    op=mybir.AluOpType.add,
                )
                nc.vector.tensor_tensor(
                    out=h1[:, 4:8, :], in0=a_s[:, 8:16:2, :], in1=a_s[:, 9:16:2, :],
                    op=mybir.AluOpType.add,
                )
                # level 2 (Pool): pairs (0,1),(2,3) -> +; (4,5),(6,7) -> -
                h2 = hpool.tile([P, 4, S], bf, tag="h2")
                nc.gpsimd.tensor_tensor(
                    out=h2[:], in0=h1[:, 0:8:2, :], in1=h1[:, 1:8:2, :],
                    op=mybir.AluOpType.add,
                )
                # level 3 (Pool): -> h3[0] = +sum(first 8 m), h3[1] = -sum(last 8 m)
                h3 = hpool.tile([P, 2, S], bf, tag="h3")
                nc.gpsimd.tensor_tensor(
                    out=h3[:], in0=h2[:, 0:4:2, :], in1=h2[:, 1:4:2, :],
                    op=mybir.AluOpType.add,
                )
                # level 4: Tm = sum_d m_d = h3[0] - h3[1]
                tm = hpool.tile([P, S], bf, tag="tm")
                nc.vector.tensor_tensor(
                    out=tm[:], in0=h3[:, 0, :], in1=h3[:, 1, :],
                    op=mybir.AluOpType.subtract,
                )
                # ec = sum_d e - sum_d c
                ec = hpool.tile([P, S], bf, tag="ec")
                nc.vector.tensor_scalar(
                    out=ec[:], in0=embE[:], scalar1=csum[:, s : s + 1],
                    scalar2=None, op0=mybir.AluOpType.subtract,
                )
                # dist = ec - 2*Tm
                dist = hpool.tile([P, S], bf, tag="dist")
                nc.vector.scalar_tensor_tensor(
                    out=dist[:], in0=tm[:], scalar=-2.0, in1=ec[:],
                    op0=mybir.AluOpType.mult, op1=mybir.AluOpType.add,
                )
                scr = spool.tile([P, S], bf, tag="scr")
                nc.scalar.activation(
                    out=scr[:],
                    in_=dist[:],
                    func=mybir.ActivationFunctionType.Square,
                    accum_out=pacc[:, s : s + 1],
                )
            nc.sync.dma_start(out=pacc_d.ap(), in_=pacc[:])
    nc.compile()
    return nc


def _get(name, param):
    key = (name, param)
    if key not in _built:
        _built[key] = _build_a(param) if name == "A" else _build_b(param)
    return _built[key]


# --------------------------------------------------------------------------
# host-side layout / prep
# --------------------------------------------------------------------------

def _round_up(x, m):
    return ((x + m - 1) // m) * m


def _split_shares(cnt):
    """Even split of cnt items over NCORES: list of per-core counts."""
    base, rem = divmod(int(cnt), NCORES)
    return [base + (1 if c < rem else 0) for c in range(NCORES)]


def _prep_layouts(emb_flat, lab_flat):
    """Build both device layouts + all bookkeeping from the raw inputs."""
    order = np.argsort(lab_flat, kind="stable")  # label-major; index (thus b) asc
    cnt_k = np.bincount(lab_flat, minlength=K)
    b_of = (np.arange(NPIX_TOT) // (H * W)).astype(np.int64)
    comb = lab_flat.astype(np.int64) * B + b_of  # label-major, b-minor == order
    cnt_kb = np.bincount(comb, minlength=K * B).reshape(K, B)  # [k, b]

    emb_bf = emb_flat.astype(BF16)

    # ---- launch B layout: partition 4k+r, d-major ----
    sharesB = {k: _split_shares(cnt_k[k]) for k in range(K)}
    maxshareB = max(max(v) for v in sharesB.values())
    S = max(_round_up(_round_up(maxshareB, 4) // 4, 16), 64)
    emb2 = np.zeros((NCORES, P, D, S), dtype=BF16)
    npad = np.zeros((NCORES, K), dtype=np.int64)
    k_starts = np.concatenate([[0], np.cumsum(cnt_k)])
    for k in range(K):
        blk = order[k_starts[k] : k_starts[k + 1]]
        off = 0
        for c in range(NCORES):
            n = sharesB[k][c]
            npad[c, k] = 4 * S - n
            if n == 0:
                continue
            arr = emb_bf[blk[off : off + n]]  # [n, D]
            off += n
            buf = np.zeros((4 * S, D), dtype=BF16)
            buf[:n] = arr
            emb2[c, 4 * k : 4 * k + 4] = buf.reshape(4, S, D).transpose(0, 2, 1)
    embE = emb2.astype(np.float32).sum(axis=2).astype(BF16)  # [8, 128, S]

    # ---- launch A layout: partition 16j+d, segments (q,b) ----
    sharesA = np.zeros((K, B, NCORES), dtype=np.int64)
    for k in range(K):
        for b in range(B):
            sharesA[k, b] = _split_shares(cnt_kb[k, b])
    maxshareA = int(sharesA.max())
    SbA = max(_round_up(maxshareA, 16), 32)
    embA = np.zeros((NCORES, P, 16 * SbA), dtype=BF16)
    kb_starts = np.concatenate([[0], np.cumsum(cnt_kb.reshape(-1))])
    for k in range(K):
        j, q = k % 8, k // 8
        for b in range(B):
            blk = order[kb_starts[k * B + b] : kb_starts[k * B + b + 1]]
            seg = 4 * q + b
            off = 0
            for c in range(NCORES):
                n = sharesA[k, b, c]
                if n == 0:
                    continue
                arr = emb_bf[blk[off : off + n]]  # [n, D]
                off += n
                embA[c, 16 * j : 16 * j + 16, seg * SbA : seg * SbA + n] = arr.T
    return dict(
        S=S, SbA=SbA, npad=npad, cnt_kb=cnt_kb,
        embA=embA, emb2=emb2, embE=embE,
    )


def _reduce_a(results, L):
    """accA [8][128,16] -> cents [B,K,D] f64 (+ f32 copy)."""
    acc = np.zeros((P, 16), dtype=np.float64)
    for c in range(NCORES):
        acc += results[c]["accA"].astype(np.float64)
    arr = acc.reshape(8, 16, 4, 4)  # [j, d, q, b]
    sums = arr.transpose(3, 2, 0, 1).reshape(B, K, D)  # [b, (q,j)=k, d]
    cnt = L["cnt_kb"].T.astype(np.float64)  # [b, k]
    cents = np.where(
        cnt[:, :, None] > 0, sums / np.maximum(cnt, 1.0)[:, :, None], 0.0
    )
    return cents


def _prep_b_smalls(cents32):
    """centq [128,64] f32 (c per (s,d)) and csum [128,4] f32 (sum_d c)."""
    kk = np.arange(P) // 4  # label of partition
    centq = np.zeros((P, 64), dtype=np.float32)
    csum = np.zeros((P, 4), dtype=np.float32)
    for s in range(4):
        centq[:, 16 * s : 16 * s + 16] = cents32[s][kk]  # [128, 16]
        csum[:, s] = cents32[s][kk].sum(-1)
    return centq, csum


def _push_host(cents):
    """Exact push term per sample from centroids (f64)."""
    dmat = np.abs(cents[:, :, None, :] - cents[:, None, :, :]).sum(-1)  # [B,K,K]
    marg = np.maximum(PUSH_MARGIN - dmat, 0.0)
    iu = np.triu_indices(K, k=1)
    return np.array([(marg[s][iu] ** 2).sum() / NCMP for s in range(B)])


def _pad_correction(cents32, csum_sk, npad):
    """Exact removal of zero-pad slots' contribution (e == 0), replicating the
    device's min-trick bf16 pipeline step by step."""
    def pair(x):
        return (x[..., 0::2].astype(np.float32) + x[..., 1::2].astype(np.float32)).astype(BF16)

    c = cents32                                            # [s, k, 16] f32
    a = np.empty_like(c)
    a[..., :8] = np.minimum(0.0 - c[..., :8], 0.0)         # DVE rows: m
    a[..., 8:] = np.maximum(c[..., 8:], 0.0)               # ACT rows: q = relu(c)
    a = a.astype(BF16)
    h1 = pair(a)                                           # [s,k,8]: 0-3 +, 4-7 -
    h2 = pair(h1)                                          # [s,k,4]: 0-1 +, 2-3 -
    h3 = pair(h2)                                          # [s,k,2]: [+, -]
    tm = (h3[..., 0].astype(np.float32) - h3[..., 1].astype(np.float32)).astype(BF16)
    ec = (0.0 - csum_sk).astype(BF16)                      # [s, k]
    dist = (tm.astype(np.float32) * -2.0 + ec.astype(np.float32)).astype(BF16)
    padtot = npad.sum(axis=0).astype(np.float64)           # [k]
    return (dist.astype(np.float64) ** 2 * padtot[None, :]).sum(-1)  # [s]


# --------------------------------------------------------------------------
# orchestration
# --------------------------------------------------------------------------

def run_launches(embeddings, labels, trace=False, trace_kwargs=None):
    emb_flat = np.ascontiguousarray(np.asarray(embeddings), dtype=F32).reshape(
        NPIX_TOT, D
    )
    lab_flat = np.ascontiguousarray(np.asarray(labels), dtype=np.int32).reshape(
        NPIX_TOT
    )
    L = _prep_layouts(emb_flat, lab_flat)
    core_ids = list(range(NCORES))
    kw = dict(trace=trace, **(trace_kwargs or {}))

    in_a = [{"embA": L["embA"][c]} for c in core_ids]
    resA = run_bass_kernel_spmd(_get("A", L["SbA"]), in_a, core_ids, **kw)
    cents = _reduce_a(resA.results, L)
    cents32 = cents.astype(np.float32)

    centq, csum = _prep_b_smalls(cents32)
    in_b = [
        {
            "emb2": L["emb2"][c].reshape(P, 16 * L["S"]),
            "embE": L["embE"][c],
            "centq": centq,
            "csum": csum,
        }
        for c in core_ids
    ]
    resB = run_bass_kernel_spmd(_get("B", L["S"]), in_b, core_ids, **kw)

    dev = np.zeros(4, dtype=np.float64)
    for c in core_ids:
        dev += resB.results[c]["pacc"].astype(np.float64).sum(axis=0)
    csum_sk = cents32.sum(-1)  # [s, k] f32, same values the device sees
    pull = (dev - _pad_correction(cents32, csum_sk, L["npad"])) / NPIX_TOT

    push = _push_host(cents)
    loss = np.mean(PUSH_W * push + PULL_W * pull)
    return np.array(loss, dtype=F32), resA, resB


def kernel(embeddings, labels):
    loss, _, _ = run_launches(embeddings, labels, trace=False)
    return loss
